# revision 14
# baseline (speedup 1.0000x reference)
"""Block-diagonal linear (grouped GEMM) on 8 TRN2 NeuronCores.

out[b, g*512+n] = sum_k x[b, g*512+k] * blocks[g, k, n]

Sharding: group-parallel — core g computes block g's GEMM. The host hands
each core xT = x[:, g*512:(g+1)*512].T ([512, 8192], feature-major) cast to
bf16 and receives outT ([512, 8192], bf16); transposes/casts happen on the
host so the device needs no PE transposes and every DMA stream reads/writes
long contiguous runs per partition.

bf16 everywhere: matmul runs 1 col/cycle at the full 2.4 GHz PE clock
(512-col cadence ~213ns measured) while halving HBM traffic vs fp32;
fp32 PSUM accumulation keeps rel err ~4e-3, inside the 2e-2 gate.
PE floor is 131072 cols / 2.4 GHz = 54.6us/core.

Per-core kernel: out.T = W.T @ x.T as 64 PSUM accumulation groups:
psum[n-tile 128, m 512] += W[k-tile, n-tile].T @ xT[k-tile, m-chunk].
Warm-up matmuls on a zeroed tile ramp the PE p-state while the first
DMAs are in flight; PSUM drains are split across DVE and ACT.
"""
import numpy as np
import ml_dtypes

import concourse.bacc as bacc
import concourse.tile as tile
from concourse import mybir
from concourse.bass_utils import run_bass_kernel_spmd

TOKENS = 8192
G = 8
M = 512  # per-block in-features
N = 512  # per-block out-features
P = 128
KT = M // P  # 4 contraction tiles
NT = N // P  # 4 output feature tiles
SUB = 512    # tokens per PSUM group (moving-dim max)
F32 = mybir.dt.float32
BF16 = mybir.dt.bfloat16
NP_BF16 = ml_dtypes.bfloat16

# chunk 0 (C0 tokens) arrives pre-packed with W in the head tensor (one
# 128-descriptor DMA, ~10us); later chunks stream per-k-tile on the HWDGE
# rings; small tail for a quick final flush
C0 = 512
CHUNKS = [C0, 512, 1024, 2048, 2048, 1024, 768, 256]
assert sum(CHUNKS) == TOKENS
CMAX = max(CHUNKS)

_CACHE: dict = {}


def _body(tc, nc, head, xT, outT):
    with (
        tc.tile_pool(name="wp", bufs=1) as wp,
        tc.tile_pool(name="xin", bufs=20) as xin,
        tc.tile_pool(name="outp", bufs=2) as outp,
        tc.tile_pool(name="pso", bufs=8, space="PSUM") as pso,
    ):
        # one 128-descriptor DMA carries W [p, kt, :N] and chunk 0 [p, kt, N:]
        head_t = wp.tile([P, KT, N + C0], BF16, tag="head")
        nc.sync.dma_start(head_t[:], head)
        w_t = head_t[:, :, :N]
        x0_t = head_t[:, :, N:]

        # Warm-up: junk matmuls on a zeroed tile ramp the PE p-state while
        # the first DMAs are still in flight (no DMA dependency).
        warm = wp.tile([P, SUB], BF16, tag="warm")
        nc.vector.memset(warm[:], 0.0)
        ps_w = pso.tile([P, SUB], F32, tag="pso")
        for _ in range(8):
            nc.tensor.matmul(ps_w[:], warm[:, :P], warm[:], start=True, stop=True)

        # cast engines per n-tile: DVE x2 + ACT x2 (GPSIMD cannot read PSUM)
        def cast_v(dst, src):
            nc.vector.tensor_copy(dst, src)

        def cast_s(dst, src):
            nc.scalar.copy(dst, src)

        cast_eng = [cast_v, cast_s, cast_v, cast_s]

        m0 = 0
        for ci, c in enumerate(CHUNKS):
            if ci == 0:
                xs = [x0_t[:, j, :] for j in range(KT)]
            else:
                # later chunks stream per-k-tile on the two HWDGE rings
                xs = []
                for j in range(KT):
                    x_t = xin.tile([P, CMAX], BF16, tag="x")
                    eng = nc.sync if j % 2 == 0 else nc.scalar
                    eng.dma_start(x_t[:, :c], xT[j * P:(j + 1) * P, m0:m0 + c])
                    xs.append(x_t[:, :])

            ots = [outp.tile([P, CMAX], BF16, tag=f"o{nt}", name=f"ot{nt}") for nt in range(NT)]
            for s0 in range(0, c, SUB):
                sw = min(SUB, c - s0)
                for nt in range(NT):
                    ps_o = pso.tile([P, SUB], F32, tag="pso")
                    for j in range(KT):
                        nc.tensor.matmul(
                            ps_o[:, :sw],
                            w_t[:, j, nt * P:(nt + 1) * P],
                            xs[j][:, s0:s0 + sw],
                            start=(j == 0),
                            stop=(j == KT - 1),
                        )
                    cast_eng[nt](ots[nt][:, s0:s0 + sw], ps_o[:, :sw])
            # flush the chunk: one DMA per n-tile on the SWDGE ring; the last
            # chunk rides the HWDGE rings (input traffic is done by then)
            for nt in range(NT):
                if ci == len(CHUNKS) - 1:
                    eng = nc.sync if nt % 2 == 0 else nc.scalar
                else:
                    eng = nc.gpsimd
                eng.dma_start(outT[nt * P:(nt + 1) * P, m0:m0 + c], ots[nt][:, :c])
            m0 += c


def _build():
    nc = bacc.Bacc("TRN2", target_bir_lowering=False, debug=False, num_devices=G)
    head = nc.dram_tensor("head", [P, KT * (N + C0)], BF16, kind="ExternalInput").ap()
    xT = nc.dram_tensor("xT", [M, TOKENS], BF16, kind="ExternalInput").ap()
    outT = nc.dram_tensor("outT", [N, TOKENS], BF16, kind="ExternalOutput").ap()
    with tile.TileContext(nc) as tc:
        _body(tc, nc, head, xT, outT)
    nc.compile()
    return nc


def _run(in_maps, **kwargs):
    if "nc" not in _CACHE:
        _CACHE["nc"] = _build()
    return run_bass_kernel_spmd(_CACHE["nc"], in_maps, list(range(G)), **kwargs)


def _in_maps(x, blocks):
    maps = []
    for g in range(G):
        xTg = np.ascontiguousarray(x[:, g * M:(g + 1) * M].T).astype(NP_BF16)
        w = blocks[g].astype(NP_BF16)
        wt = w.reshape(KT, P, N).transpose(1, 0, 2)
        x0 = xTg[:, :C0].reshape(KT, P, C0).transpose(1, 0, 2)
        head = np.concatenate([wt, x0], axis=2).reshape(P, KT * (N + C0))
        maps.append({"head": np.ascontiguousarray(head), "xT": xTg})
    return maps


def kernel(x, blocks):
    x = np.asarray(x, dtype=np.float32)
    blocks = np.asarray(blocks, dtype=np.float32)
    res = _run(_in_maps(x, blocks))
    return np.concatenate(
        [res.results[g]["outT"].T for g in range(G)], axis=1
    ).astype(np.float32)
